# revision 73
# baseline (speedup 1.0000x reference)
"""DiffAttn kernel for 8 Trainium2 NeuronCores.

Sharding: core c -> (batch b = c//2, query-half h = c%2). Each core computes
2048 query rows of both score matrices against the full K/V of its batch.
K/V projections for the full batch are computed on BOTH cores of a pair
(duplicated) -- this removes the pair AllGather entirely, which costs far
more than the duplicated projection work.

Precision strategy (rel-err budget 2e-2, achieved ~1e-3):
  * X and the (32x-scaled) weights are split on the host into fp8-e4m3
    hi + lo parts; projections run as 3 accumulating fp8 DoubleRow passes
    (Xh*Wh + Xl*Wh + Xh*Wl) at 4x fp16 PE throughput -> net 0.75x cost,
    fp16-grade accuracy.
  * K.T / Q.T are re-split on device into fp8 hi + lo (ACT writes hi from
    PSUM, DVE writes lo residual); scores run as 3 accumulating fp8
    DoubleRow passes (Kh*Qh + Kh*Ql + Kl*Qh) sharing one PSUM chain.
  * exp() via ACT with fp16 E tiles; both softmaxes are combined BEFORE
    P@V (D = E1 - (lam*s1/s2)*E2 computed in-place on DVE), so only one
    fp16 P@V pass is needed; 1/s1 applied after P@V.
"""

import math
import os

import numpy as np
import ml_dtypes

import concourse.bacc as bacc
import concourse.mybir as mybir
import concourse.tile as tile
from concourse import bass_isa
from concourse.bass_utils import run_bass_kernel_spmd

F32 = mybir.dt.float32
F16 = mybir.dt.float16
F8 = mybir.dt.float8e4
NP_F8 = ml_dtypes.float8_e4m3
AF = mybir.ActivationFunctionType
ALU = mybir.AluOpType
DR = mybir.MatmulPerfMode.DoubleRow

B, S, E, D = 4, 4096, 1024, 512
TWO_D = 2 * D
QR = S // 2          # query rows per core
QB = 512             # query block in attention
P = 128
N_EP = E // 256      # 4 contraction pair-chunks over E
N_F = TWO_D // P     # 8 feature chunks for Q/K
KC = S // P          # 32 key chunks
NQS = QB // P        # 4 q sub-blocks per query block
LAMBDA_INIT = 0.05
S_SCALE = 1.0 / math.sqrt(D)
WSCALE = 32.0        # host scales W.T by this; undone at projection write

LAST_RESULTS = None


def _emit(nc, tc, ctx):
    kin = dict(kind="ExternalInput")
    # fp8 hi/lo of X[b].T  [E, S] -- full batch (shared by the pair)
    xth = nc.dram_tensor("xth", [E, S], F8, **kin).ap()
    xtl = nc.dram_tensor("xtl", [E, S], F8, **kin).ap()
    # fp8 hi/lo of 32*W.T  [E, F]
    wqh = nc.dram_tensor("wqh", [E, TWO_D], F8, **kin).ap()
    wql = nc.dram_tensor("wql", [E, TWO_D], F8, **kin).ap()
    wkh = nc.dram_tensor("wkh", [E, TWO_D], F8, **kin).ap()
    wkl = nc.dram_tensor("wkl", [E, TWO_D], F8, **kin).ap()
    wvh = nc.dram_tensor("wvh", [E, D], F8, **kin).ap()
    wvl = nc.dram_tensor("wvl", [E, D], F8, **kin).ap()
    bq = nc.dram_tensor("bq", [TWO_D, 1], F32, **kin).ap()
    bk = nc.dram_tensor("bk", [TWO_D, 1], F32, **kin).ap()
    bv = nc.dram_tensor("bv", [1, D], F32, **kin).ap()
    lam = nc.dram_tensor("lam", [1, 1], F32, **kin).ap()
    hsel = nc.dram_tensor("hsel", [1, 1], F32, **kin).ap()  # unused on device
    out = nc.dram_tensor("out", [QR, D], F32, kind="ExternalOutput").ap()

    const = ctx.enter_context(tc.tile_pool(name="const", bufs=1))
    resident = ctx.enter_context(tc.tile_pool(name="resident", bufs=1))

    def emit_consts():
        # consolidated single-DMA constant loads on the scalar queue, after
        # the startup-critical wk/x0 loads have been issued
        bqc = const.tile([P, N_F], F32, tag="bqc")
        bkc = const.tile([P, N_F], F32, tag="bkc")
        nc.scalar.dma_start(bkc[:], bk.rearrange("(c p) o -> p (c o)", p=P))
        nc.scalar.dma_start(bqc[:], bq.rearrange("(c p) o -> p (c o)", p=P))
        bv32 = const.tile([1, D], F32, tag="bv32")
        nc.scalar.dma_start(bv32[:], bv[:])
        bvb = const.tile([P, D], F32, tag="bvb")
        nc.gpsimd.partition_broadcast(bvb[:], bv32[:])

        lam32 = const.tile([1, 1], F32, tag="lam32")
        nc.scalar.dma_start(lam32[:], lam[:])
        lam_e = const.tile([1, 1], F32, tag="lam_e")
        nc.scalar.activation(lam_e[:], lam32[:], AF.Exp)
        lam_p = const.tile([1, 1], F32, tag="lam_p")
        nc.vector.tensor_scalar(lam_p[:], lam_e[:], LAMBDA_INIT, None, ALU.add)
        plam = const.tile([P, 1], F32, tag="plam")
        nc.gpsimd.partition_broadcast(plam[:], lam_p[:])

        ones_col = const.tile([P, 1], F16, tag="ones_col")
        nc.vector.memset(ones_col[:], 1.0)
        return bqc, bkc, bvb, plam, ones_col

    # ---- resident tensors ----
    kth = resident.tile([P, N_F, S], F8, tag="kth", name="kth")    # K.T hi [f, k]
    ktl = resident.tile([P, N_F, S], F8, tag="ktl", name="ktl")    # K.T lo
    qth = resident.tile([P, N_F, QR], F8, tag="qth", name="qth")   # Q.T hi [f, q]
    qtl = resident.tile([P, N_F, QR], F8, tag="qtl", name="qtl")   # Q.T lo
    vt = resident.tile([P, KC, D], F16, tag="vt", name="vt")       # V [k, d]

    # ---------------- projection phase ----------------
    import contextlib

    with contextlib.ExitStack() as pctx:
        wres = pctx.enter_context(tc.tile_pool(name="wres", bufs=1))
        xtp = pctx.enter_context(tc.tile_pool(name="xtp", bufs=3))
        ps_work = pctx.enter_context(
            tc.tile_pool(name="ps_proj", bufs=8, space="PSUM")
        )

        # weight tiles [e, eo, f] fp8
        wq8 = {
            a: wres.tile([P, E // P, TWO_D], F8, tag=f"wq{a}", name=f"wq8{a}")
            for a in "hl"
        }
        wk8 = {
            a: wres.tile([P, E // P, TWO_D], F8, tag=f"wk{a}", name=f"wk8{a}")
            for a in "hl"
        }
        wv8 = {
            a: wres.tile([P, E // P, D], F8, tag=f"wv{a}", name=f"wv8{a}")
            for a in "hl"
        }

        # Consolidated DMA per weight tensor: DRAM [E, F] -> SBUF
        # [128, E//P, F] ([p, eo, f] <- row eo*128+p).
        def wload(q, dst, src):
            q.dma_start(dst[:], src.rearrange("(eo p) f -> p eo f", p=P))

        # K weights and X chunks load per ep-PAIR into tile sub-regions so
        # the first matmul of a chain waits only on its own contraction pair
        # (~0.7us of data) -- region-level deps let the chain chase the DMAs.
        def wload_ep(q, dst, src, ep):
            q.dma_start(
                dst[:, 2 * ep : 2 * ep + 2, :],
                src.rearrange("(eo p) f -> p eo f", p=P)[:, 2 * ep : 2 * ep + 2],
            )

        def load_x_chunk(sc):
            """[e, eo, 512] fp8 hi+lo for s-cols sc*512.. (one DMA each)."""
            xh = xtp.tile([P, E // P, 512], F8, tag="xh")
            xl = xtp.tile([P, E // P, 512], F8, tag="xl")
            nc.sync.dma_start(
                xh[:],
                xth.rearrange("(eo p) s -> p eo s", p=P)[
                    :, :, sc * 512 : (sc + 1) * 512
                ],
            )
            nc.gpsimd.dma_start(
                xl[:],
                xtl.rearrange("(eo p) s -> p eo s", p=P)[
                    :, :, sc * 512 : (sc + 1) * 512
                ],
            )
            return xh, xl

        # interleave K-weight and x0 pair loads so the first chain's ep0
        # operands land first on both queues
        first_xh = xtp.tile([P, E // P, 512], F8, tag="xh", name="xh0")
        first_xl = xtp.tile([P, E // P, 512], F8, tag="xl", name="xl0")
        xth_r0 = xth.rearrange("(eo p) s -> p eo s", p=P)
        xtl_r0 = xtl.rearrange("(eo p) s -> p eo s", p=P)
        for ep in range(N_EP):
            eos = slice(2 * ep, 2 * ep + 2)
            wload_ep(nc.sync, wk8["h"], wkh, ep)
            wload_ep(nc.gpsimd, wk8["l"], wkl, ep)
            nc.sync.dma_start(first_xh[:, eos, :], xth_r0[:, eos, 0:512])
            nc.gpsimd.dma_start(first_xl[:, eos, :], xtl_r0[:, eos, 0:512])
        bqc, bkc, bvb, plam, ones_col = emit_consts()
        # remaining weights ride sync/gpsimd behind the critical loads --
        # NOT the scalar queue, where the DMA issue would hold the ACT
        # sequencer and stall the first projection writes
        wload(nc.sync, wv8["h"], wvh)
        wload(nc.gpsimd, wv8["l"], wvl)
        wload(nc.sync, wq8["h"], wqh)
        wload(nc.gpsimd, wq8["l"], wql)

        def proj_passes(ps, wtiles, xh, xl, fsl, msl):
            """3-pass hi/lo fp8 DoubleRow projection into one PSUM chain.
            fsl: stationary free slice of W tiles; msl: moving slice of X."""
            passes = [(wtiles["h"], xh), (wtiles["h"], xl), (wtiles["l"], xh)]
            n = 0
            for ep in range(N_EP):
                for wt, xt in passes:
                    nc.tensor.matmul(
                        ps,
                        wt[:, 2 * ep : 2 * ep + 2, fsl],
                        xt[:, 2 * ep : 2 * ep + 2, msl],
                        start=n == 0,
                        stop=n == 3 * N_EP - 1,
                        perf_mode=DR,
                    )
                    n += 1

        def vproj_passes(ps, xh, xl, ssl):
            passes = [(xh, wv8["h"]), (xl, wv8["h"]), (xh, wv8["l"])]
            n = 0
            for ep in range(N_EP):
                for xt, wt in passes:
                    nc.tensor.matmul(
                        ps,
                        xt[:, 2 * ep : 2 * ep + 2, ssl],
                        wt[:, 2 * ep : 2 * ep + 2, :],
                        start=n == 0,
                        stop=n == 3 * N_EP - 1,
                        perf_mode=DR,
                    )
                    n += 1

        # own half first (hx=0 == own), then peer half (K/V only)
        # own-half s-columns in kt/vt live at my_h*QR.., handled via col_base.
        # col_base for own rows: own q rows are the SAME 2048 rows for Q.
        # Global row index of chunk: own half => my rows; peer half => other.
        # We place K/V columns by GLOBAL row index; hsel tells the host-side
        # in_map builder which X half to ship first -- device layout is fixed:
        # chunks 0..3 = own half rows, 4..7 = peer half rows, and the host
        # ships xth/xtl with own half first. kt columns: own rows at
        # [h*QR..] in global order -- to keep the device program identical on
        # all cores, the HOST pre-rolls X.T so that chunk c of xth is the
        # c-th 512-col block in the order (own half, peer half), and kt/vt
        # columns are written in GLOBAL order via col0 below, parameterized
        # only by data layout (host-side), not program structure: we store
        # K/V in ROLLED order too (own half at columns 0..QR-1), and the
        # attention loop simply consumes keys in rolled order -- softmax sums
        # and P@V are order-invariant over keys.
        for sc in range(S // 512):
            xh, xl = (first_xh, first_xl) if sc == 0 else load_x_chunk(sc)
            # K projection: out tiles [f 128, s 512]
            for fo in range(N_F):
                ps = ps_work.tile([P, 512], F32, tag="work")
                proj_passes(
                    ps[:], wk8, xh, xl,
                    slice(fo * P, (fo + 1) * P), slice(None),
                )
                nc.scalar.activation(
                    kth[:, fo, sc * 512 : (sc + 1) * 512], ps[:],
                    AF.Identity, bias=bkc[:, fo : fo + 1], scale=1.0 / WSCALE,
                )
                nc.vector.scalar_tensor_tensor(
                    ktl[:, fo, sc * 512 : (sc + 1) * 512], ps[:],
                    1.0 / WSCALE, kth[:, fo, sc * 512 : (sc + 1) * 512],
                    ALU.mult, ALU.subtract,
                )
            # V projection: out tiles [s 128, d 512]
            for ss in range(4):
                kc = sc * 4 + ss
                ps = ps_work.tile([P, 512], F32, tag="work")
                vproj_passes(ps[:], xh, xl, slice(ss * P, (ss + 1) * P))
                nc.vector.scalar_tensor_tensor(
                    vt[:, kc, :], ps[:], 1.0 / WSCALE, bvb[:],
                    ALU.mult, ALU.add,
                )
            # Q projection over own rows only (chunks 0..3 in rolled order)
            if sc < QR // 512:
                for fo in range(N_F):
                    ps = ps_work.tile([P, 512], F32, tag="work")
                    proj_passes(
                        ps[:], wq8, xh, xl,
                        slice(fo * P, (fo + 1) * P), slice(None),
                    )
                    nc.scalar.activation(
                        qth[:, fo, sc * 512 : (sc + 1) * 512], ps[:],
                        AF.Identity, bias=bqc[:, fo : fo + 1], scale=1.0 / WSCALE,
                    )
                    nc.vector.scalar_tensor_tensor(
                        qtl[:, fo, sc * 512 : (sc + 1) * 512], ps[:],
                        1.0 / WSCALE, qth[:, fo, sc * 512 : (sc + 1) * 512],
                        ALU.mult, ALU.subtract,
                    )

    # ---------------- attention phase ----------------
    with contextlib.ExitStack() as actx:
        ps_work = actx.enter_context(
            tc.tile_pool(name="ps_att", bufs=4, space="PSUM")
        )
        ps_out = actx.enter_context(tc.tile_pool(name="ps_out", bufs=4, space="PSUM"))
        e1p = actx.enter_context(tc.tile_pool(name="e1p", bufs=KC + 1))
        e2p = actx.enter_context(tc.tile_pool(name="e2p", bufs=KC))
        accp = actx.enter_context(tc.tile_pool(name="accp", bufs=2))
        srp = actx.enter_context(tc.tile_pool(name="srp", bufs=2))
        cbp = actx.enter_context(tc.tile_pool(name="cbp", bufs=2))
        rp = actx.enter_context(tc.tile_pool(name="rp", bufs=2))
        finp = actx.enter_context(tc.tile_pool(name="finp", bufs=2))

        def score_block(qb, m, split_last_kc=False):
            """Scores for query block qb, matrix m -> E tiles + row-sum acc.
            split_last_kc: emit the final kc chain in column halves so the
            softmax-combine chain for h0 starts before the h1 matmuls end
            (used on the very last block, where nothing else hides it)."""
            ep = e1p if m == 0 else e2p
            acc_t = accp.tile([P, QB], F16, tag="acc", name=f"acc_{qb}_{m}")
            e_tiles = []
            passes = [(kth, qth), (kth, qtl), (ktl, qth)]

            def chain(ps, kc, q0, w, psl):
                n = 0
                for fp_ in range(2):
                    fo = 4 * m + 2 * fp_
                    for kt_, qt_ in passes:
                        nc.tensor.matmul(
                            ps[:, psl],
                            kt_[:, fo : fo + 2, kc * P : (kc + 1) * P],
                            qt_[:, fo : fo + 2, q0 : q0 + w],
                            start=n == 0,
                            stop=n == 5,
                            perf_mode=DR,
                        )
                        n += 1

            for kc in range(KC):
                ps = ps_work.tile([P, QB], F32, tag="work")
                et = ep.tile([P, QB], F16, tag="e")
                # the matmul chain is always ONE full-width accumulation
                # group (sub-region PSUM chains crash the executor); only
                # the exp READ + acc-add are split on the final tile
                chain(ps, kc, qb * QB, QB, slice(None))
                halves = (
                    ((0, QB),)
                    if not (split_last_kc and kc == KC - 1)
                    else ((0, QB // 2), (QB // 2, QB // 2))
                )
                for c0, w in halves:
                    sl = slice(c0, c0 + w)
                    nc.scalar.activation(et[:, sl], ps[:, sl], AF.Exp, scale=S_SCALE)
                    if kc == 0:
                        nc.vector.tensor_copy(acc_t[:, sl], et[:, sl])
                    else:
                        nc.vector.tensor_tensor(
                            acc_t[:, sl], acc_t[:, sl], et[:, sl], ALU.add
                        )
                e_tiles.append(et)
            return acc_t, e_tiles

        def sums_m0(acc0):
            """s1 broadcast row-sums + reciprocal column sums for 1/s1."""
            s1b = srp.tile([P, QB], F16, tag="sr")
            nc.gpsimd.partition_all_reduce(s1b[:], acc0[:], 128, bass_isa.ReduceOp.add)
            # borrow a ps_out slot (same tag => shares the 4 round-robin
            # slots; the oldest slot is freed by the previous block's first
            # fin long before this sums result is needed)
            sums_full = ps_out.tile([P, D], F32, tag="out", name="sums_slot")
            sums_col = sums_full[:, 0:NQS]
            for qs in range(NQS):
                nc.tensor.matmul(
                    sums_col[:, qs : qs + 1],
                    acc0[:, qs * P : (qs + 1) * P],
                    ones_col[:],
                    start=qs == 0,
                    stop=qs == NQS - 1,
                )
            r1c = rp.tile([P, NQS], F32, tag="r")
            nc.vector.reciprocal(r1c[:], sums_col[:])
            return s1b, r1c

        def combine_dt(acc1, s1b, e1t, e2t, last=False):
            """Softmax-combine chain (column halves) + in-place
            D = E1 - cb*E2 on DVE.  For the final query block the dt ops are
            emitted fully half-by-half so P@V over qs0/1 can run while the
            h1 chain is still producing (nothing else covers the chain
            there)."""
            s2b = srp.tile([P, QB], F16, tag="sr")
            cb = cbp.tile([P, QB], F16, tag="cb")
            H = QB // 2

            def dt_ops(kc, sl, eng=None):
                eng = eng or nc.vector
                eng.tensor_tensor(
                    e2t[kc][:, sl], e2t[kc][:, sl], cb[:, sl], ALU.mult
                )
                eng.tensor_tensor(
                    e1t[kc][:, sl], e1t[kc][:, sl], e2t[kc][:, sl], ALU.subtract
                )

            for h in range(2):
                sl = slice(h * H, (h + 1) * H)
                nc.gpsimd.partition_all_reduce(
                    s2b[:, sl], acc1[:, sl], 128, bass_isa.ReduceOp.add
                )
                with nc.allow_low_precision(reason="f16 recip of row sums"):
                    nc.vector.reciprocal(s2b[:, sl], s2b[:, sl])
                nc.vector.scalar_tensor_tensor(
                    cb[:, sl], s2b[:, sl], plam[:, 0:1], s1b[:, sl],
                    ALU.mult, ALU.mult,
                )
                if last:
                    for kc in range(KC):
                        dt_ops(kc, sl)
            if not last:
                for kc in range(KC):
                    for h in range(2) if kc < 2 else (slice(None),):
                        sl = slice(h * H, (h + 1) * H) if isinstance(h, int) else h
                        dt_ops(kc, sl)

        def pv_block(qb, e1t, r1c, last=False):
            """Single P@V pass over the combined D tiles, then normalize.
            For the final block: process qs pairs per column half so the
            first half's chains (and output writeback) overlap the second
            half's dt production."""
            outp = [
                ps_out.tile([P, D], F32, tag="out", name=f"out{qb}_{qs}")
                for qs in range(NQS)
            ]
            out_qs = [nc.gpsimd, nc.sync, nc.gpsimd, nc.sync]

            def emit_fin(qs):
                fin = finp.tile([P, D], F32, tag="fin")
                if qs % 2:
                    # odd qs normalize on the (idle) ACT engine so the two
                    # fins of a half run concurrently at the kernel tail
                    nc.scalar.activation(
                        fin[:], outp[qs][:], AF.Identity, scale=r1c[:, qs : qs + 1]
                    )
                else:
                    nc.vector.tensor_scalar(
                        fin[:], outp[qs][:], r1c[:, qs : qs + 1], None, ALU.mult
                    )
                row0 = qb * QB + qs * P
                out_qs[qs].dma_start(out[row0 : row0 + P, :], fin[:])

            if last:
                for half in range(2):
                    qss = (2 * half, 2 * half + 1)
                    for kc in range(KC):
                        for qs in qss:
                            nc.tensor.matmul(
                                outp[qs][:],
                                e1t[kc][:, qs * P : (qs + 1) * P],
                                vt[:, kc, :],
                                start=kc == 0,
                                stop=kc == KC - 1,
                            )
                    for qs in qss:
                        emit_fin(qs)
            else:
                for kc in range(KC):
                    for qs in range(NQS):
                        nc.tensor.matmul(
                            outp[qs][:],
                            e1t[kc][:, qs * P : (qs + 1) * P],
                            vt[:, kc, :],
                            start=kc == 0,
                            stop=kc == KC - 1,
                        )
                for qs in range(NQS):
                    emit_fin(qs)

        # software pipeline over query blocks: emit next block's m0 scores
        # before this block's P@V so PE never stalls on the DVE combine.
        NQB = QR // QB
        acc0, e1t = score_block(0, 0)
        s1b, r1c = sums_m0(acc0)
        acc1, e2t = score_block(0, 1)
        state = (e1t, e2t, s1b, r1c, acc1)
        for qb in range(NQB):
            e1t, e2t, s1b, r1c, acc1 = state
            last = qb + 1 >= NQB
            combine_dt(acc1, s1b, e1t, e2t, last=last)
            if not last:
                acc0_n, e1t_n = score_block(qb + 1, 0)
            pv_block(qb, e1t, r1c, last=last)
            if not last:
                s1b_n, r1c_n = sums_m0(acc0_n)
                acc1_n, e2t_n = score_block(
                    qb + 1, 1, split_last_kc=(qb + 2 >= NQB)
                )
                state = (e1t_n, e2t_n, s1b_n, r1c_n, acc1_n)


_NC_CACHE = {}


def _get_nc():
    if "nc" not in _NC_CACHE:
        nc = bacc.Bacc("TRN2", target_bir_lowering=False, debug=False, num_devices=8)
        with tile.TileContext(nc) as tc:
            with __import__("contextlib").ExitStack() as ctx:
                _emit(nc, tc, ctx)
        nc.compile()
        _NC_CACHE["nc"] = nc
    return _NC_CACHE["nc"]


def _hi_lo(a):
    hi = a.astype(NP_F8)
    lo = (a - hi.astype(np.float32)).astype(NP_F8)
    return np.ascontiguousarray(hi), np.ascontiguousarray(lo)


def build_in_maps(X, Wq, bq, Wk, bk, Wv, bv, lam, **_unused):
    X = np.asarray(X, dtype=np.float32)
    WqT = np.asarray(Wq, dtype=np.float32).T * WSCALE  # [E, 2D]
    WkT = np.asarray(Wk, dtype=np.float32).T * WSCALE
    WvT = np.asarray(Wv, dtype=np.float32).T * WSCALE  # [E, D]
    wqh, wql = _hi_lo(WqT)
    wkh, wkl = _hi_lo(WkT)
    wvh, wvl = _hi_lo(WvT)
    bq_ = np.ascontiguousarray(np.asarray(bq, dtype=np.float32).reshape(TWO_D, 1))
    bk_ = np.ascontiguousarray(np.asarray(bk, dtype=np.float32).reshape(TWO_D, 1))
    bv_ = np.ascontiguousarray(np.asarray(bv, dtype=np.float32).reshape(1, D))
    lam_ = np.ascontiguousarray(np.asarray(lam, dtype=np.float32).reshape(1, 1))

    in_maps = []
    for c in range(8):
        b, h = c // 2, c % 2
        # X.T for the full batch, rolled so this core's own 2048 rows come
        # first (the kernel projects Q from chunks 0..3 and stores K/V in
        # rolled key order -- softmax and P@V are key-order invariant).
        xt = X[b].T  # [E, S]
        if h == 1:
            xt = np.concatenate([xt[:, QR:], xt[:, :QR]], axis=1)
        xh, xl = _hi_lo(np.ascontiguousarray(xt))
        in_maps.append(
            {
                "xth": xh, "xtl": xl,
                "wqh": wqh, "wql": wql,
                "wkh": wkh, "wkl": wkl,
                "wvh": wvh, "wvl": wvl,
                "bq": bq_, "bk": bk_, "bv": bv_, "lam": lam_,
                "hsel": np.full((1, 1), float(h), np.float32),
            }
        )
    return in_maps


def kernel(X, Wq, bq, Wk, bk, Wv, bv, lam, **_unused):
    global LAST_RESULTS
    nc = _get_nc()
    in_maps = build_in_maps(X, Wq, bq, Wk, bk, Wv, bv, lam)
    trace = bool(int(os.environ.get("DIFFATTN_TRACE", "0")))
    res = run_bass_kernel_spmd(nc, in_maps, core_ids=list(range(8)), trace=trace)
    LAST_RESULTS = res
    full = np.empty((B, S, D), dtype=np.float32)
    for c in range(8):
        b, h = c // 2, c % 2
        full[b, h * QR : (h + 1) * QR] = res.results[c]["out"]
    return full
